# revision 6
# baseline (speedup 1.0000x reference)
"""Trainium2 Bass kernel for nn_CausalAttentionPooling.

Math: scores[b,i,j] = x[b,i].q are constant along the softmax axis j, so
softmax over the causal mask yields uniform weights 1/(i+1) on j <= i.
The module is exactly a causal cumulative mean:
    out[b,i,:] = cumsum(x, axis=1)[b,i,:] / (i+1)
(q does not affect the output.)

Sharding: 8 shards = (batch b in 0..3) x (D-half dh in 0..1); each core gets
x[b, :, dh*128:(dh+1)*128] transposed to [128(D), 4096(L)], shipped as bf16
(halves input DMA; scan state stays fp32 so only input rounding ~2^-9 rel).

Per core:
  - PE: 8 bf16 outer products ones[1,128] @ rrow[1,512] -> PSUM banks hold
    1/(i+1) replicated across partitions (bf16 matmul: 1 cyc/row).
  - Act: copies scale banks 0..5 PSUM->SBUF (GPSIMD can't read PSUM).
  - DVE: chained tensor_tensor_scan over all 4096 cols (bf16 in, fp32 out).
  - Pool: finalize mult chunks 0..5 (cum * rr_sb) trailing the scans.
  - DVE: finalize chunks 6,7 (reads rr straight from PSUM) after its scans;
    last chunk is 256 wide to shorten the tail.
  - In-DMAs issued on SP (6 x 512-aligned spans); out-DMAs all on SP too
    (Act is busy with copies; SP is idle after inputs).
"""

import numpy as np

B, L, D = 4, 4096, 256
NCORES = 8
P = 128
PB = 512                 # psum bank / finalize chunk width
NB = L // PB             # 8 chunks

_cache = {}


def _split_waits_bir(bir_bytes):
    """This container's walrus build rejects instructions carrying more than
    one (or for some opcodes, two) sync waits.  Hoist multi-wait sync_info
    onto standalone same-engine EventSemaphore instructions inserted
    immediately before the instruction; program order on the engine's stream
    preserves semantics."""
    import orjson

    d = orjson.loads(bir_bytes)
    n = 0
    for fn in d["functions"]:
        for bb in fn["blocks"]:
            out = []
            for inst in bb["instructions"]:
                si = inst.get("sync_info")
                waits = (si or {}).get("on_wait") or []
                if len(waits) > 1:
                    for w in waits:
                        out.append(
                            {
                                "debug": inst.get("debug"),
                                "engine": inst["engine"],
                                "ins": [],
                                "name": f"I-waitfix-{n}",
                                "opcode": "EventSemaphore",
                                "outs": [],
                                "sync_info": {"on_wait": [w], "on_update": []},
                            }
                        )
                        n += 1
                    si["on_wait"] = []
                out.append(inst)
            bb["instructions"] = out
    return orjson.dumps(d)


def _install_bir_patch():
    if _cache.get("patched"):
        return
    import concourse.bass as bass

    orig = bass.Bass.to_json_bytes

    def patched(self):
        return _split_waits_bir(orig(self))

    bass.Bass.to_json_bytes = patched
    _cache["patched"] = True


def _build_nc():
    import concourse.bass as bass
    import concourse.tile as tile
    from concourse import mybir

    _install_bir_patch()

    f32 = mybir.dt.float32
    bf16 = mybir.dt.bfloat16
    add = mybir.AluOpType.add
    byp = mybir.AluOpType.bypass
    mult = mybir.AluOpType.mult

    nc = bass.Bass()
    xT = nc.declare_dram_parameter("xT", [P, L], bf16, isOutput=False)
    rrow = nc.declare_dram_parameter("rrow", [1, L], bf16, isOutput=False)
    out = nc.declare_dram_parameter("out", [P, L], f32, isOutput=True)

    N_POOL = 6           # finalize chunks 0..5 on Pool, rest on DVE

    with tile.TileContext(nc) as tc:
        with (
            tc.tile_pool(name="sb", bufs=1) as sb,
            tc.tile_pool(name="ps", bufs=1, space="PSUM") as ps,
        ):
            xt = sb.tile([P, L], bf16, tag="xt")
            cum = sb.tile([P, L], f32, tag="cum")
            ot = sb.tile([P, L], f32, tag="ot")
            rrow_sb = sb.tile([1, L], bf16, tag="rrow")
            ones = sb.tile([1, P], bf16, tag="ones")
            rr_sb = sb.tile([P, N_POOL * PB], f32, tag="rrsb")

            # ---- input DMAs on SP: rrow first (tiny, unblocks PE), then x
            nc.sync.dma_start(rrow_sb[:], rrow[:])
            xspans = [
                (0, 512), (512, 1024), (1024, 1536),
                (1536, 2560), (2560, 3584), (3584, 4096),
            ]
            for a, b in xspans:
                nc.sync.dma_start(xt[:, a:b], xT[:, a:b])

            # ---- ones memset on Pool (idle early), PE replicates 1/(i+1)
            nc.gpsimd.memset(ones[:], 1.0)
            rr_ps = []
            for j in range(NB):
                pt = ps.tile([P, PB], f32, tag=f"rr{j}")
                nc.tensor.matmul(
                    pt[:],
                    ones[:],
                    rrow_sb[:, j * PB : (j + 1) * PB],
                    start=True,
                    stop=True,
                )
                rr_ps.append(pt)

            # ---- Act: copy scale banks for Pool's chunks into SBUF
            for c in range(N_POOL):
                nc.scalar.copy(rr_sb[:, c * PB : (c + 1) * PB], rr_ps[c][:])

            # ---- DVE: chained scans (exact cumsum in fp32)
            for a, b in xspans:
                init = 0.0 if a == 0 else cum[:, a - 1 : a]
                nc.vector.tensor_tensor_scan(
                    cum[:, a:b], xt[:, a:b], xt[:, a:b], init, op0=add, op1=byp
                )

            # ---- finalize mults + out DMAs (issued on SP as chunks finish)
            def fin_out(eng, a, b):
                c = a // PB
                rr = (
                    rr_sb[:, a : b]
                    if c < N_POOL
                    else rr_ps[c][:, a - c * PB : b - c * PB]
                )
                eng.tensor_tensor(ot[:, a:b], cum[:, a:b], rr, op=mult)
                nc.sync.dma_start(out[:, a:b], ot[:, a:b])

            for c in range(N_POOL):
                fin_out(nc.gpsimd, c * PB, (c + 1) * PB)
            # DVE tail after its scans; last piece small to shorten the tail
            fin_out(nc.vector, 6 * PB, 7 * PB)
            fin_out(nc.vector, 7 * PB, 7 * PB + 256)
            fin_out(nc.vector, 7 * PB + 256, L)
    return nc


def _get_nc():
    if "nc" not in _cache:
        _cache["nc"] = _build_nc()
    return _cache["nc"]


def _make_in_maps(x):
    import ml_dtypes

    bf16 = ml_dtypes.bfloat16
    idx = np.arange(1, L + 1, dtype=np.float64)
    rrow = (1.0 / idx).astype(bf16).reshape(1, L)
    in_maps = []
    shards = []
    for c in range(NCORES):
        b, dh = c // 2, c % 2
        shards.append((b, dh))
        xT = np.ascontiguousarray(x[b, :, dh * P : (dh + 1) * P].T.astype(bf16))
        in_maps.append({"xT": xT, "rrow": rrow})
    return in_maps, shards


def kernel(x, q):
    from concourse.bass_utils import run_bass_kernel_spmd

    x = np.asarray(x)
    assert x.shape == (B, L, D) and x.dtype == np.float32

    nc = _get_nc()
    in_maps, shards = _make_in_maps(x)
    results = run_bass_kernel_spmd(nc, in_maps, list(range(NCORES))).results

    out = np.empty((B, L, D), dtype=np.float32)
    for c, (b, dh) in enumerate(shards):
        out[b, :, dh * P : (dh + 1) * P] = results[c]["out"].T
    return out


# revision 9
# speedup vs baseline: 1.1239x; 1.1239x over previous
"""Trainium2 Bass kernel for nn_CausalAttentionPooling.

Math: scores[b,i,j] = x[b,i].q are constant along the softmax axis j, so
softmax over the causal mask yields uniform weights 1/(i+1) on j <= i.
The module is exactly a causal cumulative mean:
    out[b,i,:] = cumsum(x, axis=1)[b,i,:] / (i+1)
(q does not affect the output.)

Sharding: 8 shards = (batch b in 0..3) x (D-half dh in 0..1); each core gets
x[b, :, dh*128:(dh+1)*128] transposed to [128(D), 4096(L)] fp32.

Measured HW rates (ntff profiles): DVE fp32 scan ~2.3 ns/col, DVE fp32
tensor_tensor ~1.29 ns/col, all-2-byte tensor_tensor hits the 2x_1p mode;
Pool tensor_tensor contends with DVE on SBUF (both ~2x slower) so Pool gets
no elementwise work.  Everything numeric-critical (the running sum) stays
fp32; only the final scale+output are bf16 (tol is 2e-2, bf16 adds ~4e-3).

Per core:
  - PE: 8 bf16 outer products ones[1,128] @ rrow[1,512] -> PSUM = 1/(i+1)
    replicated across partitions.
  - Act: copies the scale banks PSUM->SBUF as bf16 (enables DVE 2x mode).
  - DVE: chained fp32-src tensor_tensor_scan -> cum (bf16 out, fp32 state),
    interleaved with all-bf16 finalize mults (cum * rr -> ot bf16).
  - out DMA ships bf16 (1 MB/core); host upcasts to fp32.
  - In-DMAs on SP queue; out-DMAs + rr copies on Act queue.
"""

import numpy as np

B, L, D = 4, 4096, 256
NCORES = 8
P = 128
PB = 512                 # psum bank width
NB = L // PB

_cache = {}


def _split_waits_bir(bir_bytes):
    """This container's walrus build rejects instructions carrying more than
    one (or for some opcodes, two) sync waits.  Hoist multi-wait sync_info
    onto standalone same-engine EventSemaphore instructions inserted
    immediately before the instruction; program order on the engine's stream
    preserves semantics."""
    import orjson

    d = orjson.loads(bir_bytes)
    n = 0
    for fn in d["functions"]:
        for bb in fn["blocks"]:
            out = []
            for inst in bb["instructions"]:
                si = inst.get("sync_info")
                waits = (si or {}).get("on_wait") or []
                if len(waits) > 1:
                    for w in waits:
                        out.append(
                            {
                                "debug": inst.get("debug"),
                                "engine": inst["engine"],
                                "ins": [],
                                "name": f"I-waitfix-{n}",
                                "opcode": "EventSemaphore",
                                "outs": [],
                                "sync_info": {"on_wait": [w], "on_update": []},
                            }
                        )
                        n += 1
                    si["on_wait"] = []
                out.append(inst)
            bb["instructions"] = out
    return orjson.dumps(d)


def _install_bir_patch():
    if _cache.get("patched"):
        return
    import concourse.bass as bass

    orig = bass.Bass.to_json_bytes

    def patched(self):
        return _split_waits_bir(orig(self))

    bass.Bass.to_json_bytes = patched
    _cache["patched"] = True


def _build_nc():
    import concourse.bass as bass
    import concourse.tile as tile
    from concourse import mybir

    _install_bir_patch()

    f32 = mybir.dt.float32
    bf16 = mybir.dt.bfloat16
    add = mybir.AluOpType.add
    byp = mybir.AluOpType.bypass
    mult = mybir.AluOpType.mult

    nc = bass.Bass()
    xT = nc.declare_dram_parameter("xT", [P, L], f32, isOutput=False)
    rrow = nc.declare_dram_parameter("rrow", [1, L], bf16, isOutput=False)
    out = nc.declare_dram_parameter("out", [P, L], bf16, isOutput=True)

    # scan spans (chained on DVE): small first span for an early start,
    # 1024-wide middles (best ns/col), small last span for a short tail
    xspans = [(0, 128), (128, 1152), (1152, 2176), (2176, 3200),
              (3200, 3968), (3968, L)]

    with tile.TileContext(nc) as tc:
        with (
            tc.tile_pool(name="sb", bufs=1) as sb,
            tc.tile_pool(name="ps", bufs=1, space="PSUM") as ps,
        ):
            xt = sb.tile([P, L], f32, tag="xt")
            cum = sb.tile([P, L], bf16, tag="cum")
            ot = sb.tile([P, L], bf16, tag="ot")
            rrow_sb = sb.tile([1, L], bf16, tag="rrow")
            ones = sb.tile([1, P], bf16, tag="ones")
            rr_sb = sb.tile([P, L], bf16, tag="rrsb")

            # ---- input DMAs on SP: rrow first (tiny, unblocks PE), then x
            nc.sync.dma_start(rrow_sb[:], rrow[:])
            for a, b in xspans:
                nc.sync.dma_start(xt[:, a:b], xT[:, a:b])

            # ---- ones memset on Pool (idle), PE replicates 1/(i+1)
            nc.gpsimd.memset(ones[:], 1.0)
            rr_ps = []
            for j in range(NB):
                pt = ps.tile([P, PB], f32, tag=f"rr{j}")
                nc.tensor.matmul(
                    pt[:],
                    ones[:],
                    rrow_sb[:, j * PB : (j + 1) * PB],
                    start=True,
                    stop=True,
                )
                rr_ps.append(pt)

            # ---- Act: scale banks PSUM -> SBUF bf16
            for c in range(NB):
                nc.scalar.copy(rr_sb[:, c * PB : (c + 1) * PB], rr_ps[c][:])

            # ---- DVE: chained scans (fp32 src, bf16 cum out; span chaining
            # through the bf16 boundary column adds <=0.4% per hop, fine for
            # the 2e-2 tolerance) with finalize mults one span behind
            def scan(si):
                a, b = xspans[si]
                init = 0.0 if si == 0 else cum[:, a - 1 : a]
                nc.vector.tensor_tensor_scan(
                    cum[:, a:b], xt[:, a:b], xt[:, a:b], init, op0=add, op1=byp
                )

            def fin(si):
                a, b = xspans[si]
                nc.vector.tensor_tensor(
                    ot[:, a:b], cum[:, a:b], rr_sb[:, a:b], op=mult
                )
                nc.scalar.dma_start(out[:, a:b], ot[:, a:b])

            for si in range(len(xspans)):
                scan(si)
                if si >= 1:
                    fin(si - 1)
            fin(len(xspans) - 1)
    return nc


def _get_nc():
    if "nc" not in _cache:
        _cache["nc"] = _build_nc()
    return _cache["nc"]


def _make_in_maps(x):
    import ml_dtypes

    bf16 = ml_dtypes.bfloat16
    idx = np.arange(1, L + 1, dtype=np.float64)
    rrow = (1.0 / idx).astype(bf16).reshape(1, L)
    in_maps = []
    shards = []
    for c in range(NCORES):
        b, dh = c // 2, c % 2
        shards.append((b, dh))
        xT = np.ascontiguousarray(x[b, :, dh * P : (dh + 1) * P].T)
        in_maps.append({"xT": xT, "rrow": rrow})
    return in_maps, shards


def kernel(x, q):
    from concourse.bass_utils import run_bass_kernel_spmd

    x = np.asarray(x)
    assert x.shape == (B, L, D) and x.dtype == np.float32

    nc = _get_nc()
    in_maps, shards = _make_in_maps(x)
    results = run_bass_kernel_spmd(nc, in_maps, list(range(NCORES))).results

    out = np.empty((B, L, D), dtype=np.float32)
    for c, (b, dh) in enumerate(shards):
        out[b, :, dh * P : (dh + 1) * P] = results[c]["out"].T.astype(np.float32)
    return out
